# revision 1
# baseline (speedup 1.0000x reference)
"""Trainium2 Bass kernel for nn_Decoder_39324720562636.

Self-contained: hardcodes shapes/sharding. 8 NeuronCores, data-parallel:
core = 2*sample + half; each core computes a 64-row half of one sample's
128x128 output, with a pairwise AllReduce for the channel-attention Gram
matrix + norms.

Layout: channels on partitions (2 blocks of 128), pixels on free dim, fp16
data path, fp32 PSUM accumulation. LayerNorm stats via PE transposes +
bn_stats; Gram via PE-transposed Q operands; depthwise 3x3 via diagonal
matmuls; BN folded into the pointwise conv host-side.
"""
import numpy as np

import concourse.bass as bass
import concourse.bacc as bacc
import concourse.tile as tile
import concourse.mybir as mybir

dt = mybir.dt
OP = mybir.AluOpType
AF = mybir.ActivationFunctionType
AX = mybir.AxisListType
F16 = dt.float16
F32 = dt.float32

B, C, H, W, HEADS = 4, 256, 64, 64, 8
H2, W2 = 128, 128
NCORES = 8
ROWS = 66            # tile rows = image [r0-1, r0+65)
NPIX = ROWS * W2     # 8448
QPIX = 8192          # Q/Gram pixels: tile rows 1..64
QOFF = 128           # free-dim offset of Q-rows
XR = 34              # x slice rows
YW = 130             # padded y row width
NCH = 16             # 512-px chunks in Q phase


def _vchunks():
    return [(512 * k, 512) for k in range(16)] + [(8192, 256)]


# ---------------------------------------------------------------- builder --
def build(reps: int = 1, debug: bool = False, stop_after: int = 99):
    nc = bacc.Bacc("TRN2", target_bir_lowering=False, debug=False,
                   num_devices=NCORES)

    def din(name, shape, d=F16):
        return nc.dram_tensor(name, shape, d, kind="ExternalInput").ap()

    t_in = {
        "xs": din("xs", [C, XR * W]),
        "res": din("res", [C, NPIX]),
        "wq_d": din("wq_d", [2, 2, 128, 128]),
        "wq_g": din("wq_g", [2, 2, 128, 128]),
        "wv_d": din("wv_d", [2, 2, 128, 128]),
        "wv_g": din("wv_g", [2, 2, 128, 128]),
        "pw_l": din("pw_l", [2, 2, 128, 128]),
        "dwd": din("dwd", [9, 2, 128, 128]),
        "ident": din("ident", [128, 128]),
        "bones": din("bones", [4, 128]),
        "bq": din("bq", [128, 4], F32),
        "bv": din("bv", [128, 4], F32),
        "pwc": din("pwc", [128, 2], F32),
        "svec": din("svec", [128, 2], F32),
        "bgv": din("bgv", [128, 4], F32),
        "msk": din("msk", [128, 2], F32),
    }
    out_d = nc.dram_tensor("out", [C, QPIX], F32, kind="ExternalOutput").ap()
    dbg = {}
    if debug:
        for nm, sh, d in [("d_xup", [C, NPIX], F16), ("d_ln", [C, QPIX], F16),
                          ("d_q", [C, QPIX], F16), ("d_g", [C, 260], F32),
                          ("d_a", [C, 64], F16), ("d_v", [C, 2 * NPIX], F16),
                          ("d_y", [C, ROWS * YW], F16),
                          ("d_mv", [128, 256], F32)]:
            dbg[nm] = nc.dram_tensor(nm, sh, d, kind="ExternalOutput").ap()

    with tile.TileContext(nc) as tc:
        with (
            tc.tile_pool(name="const", bufs=1) as cp,
            tc.tile_pool(name="big", bufs=1) as bp,
        ):
            cst = {}
            for nm in ("wq_d", "wq_g", "wv_d", "wv_g", "pw_l"):
                for kb in range(2):
                    for ob in range(2):
                        t = cp.tile([128, 128], F16, name=f"{nm}{kb}{ob}")
                        nc.sync.dma_start(t[:], t_in[nm][kb, ob])
                        cst[(nm, kb, ob)] = t
            for s in range(9):
                for ob in range(2):
                    t = cp.tile([128, 128], F16, name=f"dw{s}{ob}")
                    nc.sync.dma_start(t[:], t_in["dwd"][s, ob])
                    cst[("dw", s, ob)] = t
            for nm, sh, d in [("ident", [128, 128], F16),
                              ("bones", [4, 128], F16), ("bq", [128, 4], F32),
                              ("bv", [128, 4], F32), ("pwc", [128, 2], F32),
                              ("svec", [128, 2], F32), ("bgv", [128, 4], F32),
                              ("msk", [128, 2], F32)]:
                t = cp.tile(sh, d, name=f"c_{nm}")
                nc.sync.dma_start(t[:], t_in[nm][:])
                cst[nm] = t

            zb = cp.tile([128, 1], F32, name="zb")
            nc.vector.memset(zb[:], 0.0)
            epsb = cp.tile([128, 1], F32, name="epsb")
            nc.vector.memset(epsb[:], 1e-6)
            cst["zb"], cst["epsb"] = zb, epsb

            for _rep in range(reps):
                _emit_rep(nc, tc, bp, cst, t_in, out_d, dbg, stop_after)

    nc.compile()
    return nc


def _emit_rep(nc, tc, bp, cst, t_in, out_d, dbg, stop_after=99):
    def _bail():
        with tc.tile_pool(name="bail", bufs=1) as bl:
            t = bl.tile([128, 512], F32, name="bailt")
            nc.vector.memset(t[:], 0.0)
            for o in range(2):
                nc.sync.dma_start(out_d[128 * o:128 * o + 128, 0:512], t[:])

    ident = cst["ident"]
    xs_in, res_in = t_in["xs"], t_in["res"]

    xup = bp.tile([128, 2, NPIX], F16, tag="xup", name="xup")
    Vd = bp.tile([128, 2, NPIX], F16, tag="Vd", name="Vd")
    Vg = bp.tile([128, 2, NPIX], F16, tag="Vg", name="Vg")
    Vsb = {"d": Vd, "g": Vg}

    # ================================================== phase 0: upsample ==
    with tc.tile_pool(name="up", bufs=1) as up:
        for b in range(2):
            xs = up.tile([128, XR, W], F16, tag="xs", name="xs", bufs=2)
            nc.sync.dma_start(xs[:], xs_in[128 * b:128 * b + 128, :])
            t75 = up.tile([128, XR, W], F16, tag="t75", name="t75", bufs=2)
            nc.vector.tensor_scalar(t75[:], xs[:], 0.75, None, OP.mult)
            xh = up.tile([128, XR, W2], F16, tag="xh", name="xh", bufs=2)
            nc.vector.tensor_copy(xh[:, :, 0], xs[:, :, 0])
            nc.vector.tensor_copy(xh[:, :, 127], xs[:, :, 63])
            # even cols 2n (n=1..63): 0.25 x[n-1] + 0.75 x[n]
            nc.vector.scalar_tensor_tensor(xh[:, :, 2:127:2], xs[:, :, 0:63],
                                           0.25, t75[:, :, 1:64],
                                           OP.mult, OP.add)
            # odd cols 2n+1 (n=0..62): 0.75 x[n] + 0.25 x[n+1]
            nc.vector.scalar_tensor_tensor(xh[:, :, 1:127:2], xs[:, :, 1:64],
                                           0.25, t75[:, :, 0:63],
                                           OP.mult, OP.add)
            v75 = up.tile([128, XR, W2], F16, tag="v75", name="v75", bufs=2)
            nc.vector.tensor_scalar(v75[:], xh[:], 0.75, None, OP.mult)
            xv = xup[:, b].rearrange("p (r w) -> p r w", w=W2)
            # tile row 2m = 0.75 xh[m] + 0.25 xh[m+1], m=0..32
            nc.vector.scalar_tensor_tensor(xv[:, 0:65:2, :], xh[:, 1:34, :],
                                           0.25, v75[:, 0:33, :],
                                           OP.mult, OP.add)
            nc.vector.scalar_tensor_tensor(xv[:, 1:66:2, :], xh[:, 0:33, :],
                                           0.25, v75[:, 1:34, :],
                                           OP.mult, OP.add)
        if "d_xup" in dbg:
            for b in range(2):
                nc.sync.dma_start(dbg["d_xup"][128 * b:128 * b + 128, :],
                                  xup[:, b])

    if stop_after < 1:
        _bail()
        return
    # =============================== phase 1: LN/Q/Gram + V convs ==========
    with tc.tile_pool(name="gps", bufs=1, space="PSUM") as psG:
        Gps = [psG.tile([128, 256], F32, tag=f"G{b}", name=f"G{b}")
               for b in range(2)]
        nacc = bp.tile([128, 2, 2, NCH], F32, tag="nacc", name="nacc")

        with (
            tc.tile_pool(name="p1ps", bufs=1, space="PSUM") as pp,
            tc.tile_pool(name="p1w", bufs=1) as wp,
        ):
            for k in range(NCH):
                off = QOFF + 512 * k
                rg = wp.tile([128, 2, 512], F16, tag="rg", name="rg", bufs=3)
                for b in range(2):
                    nc.sync.dma_start(rg[:, b],
                                      res_in[128 * b:128 * b + 128,
                                             off:off + 512])
                ln = {s: wp.tile([128, 2, 512], F16, tag=f"ln{s}",
                                 name=f"ln{s}", bufs=2) for s in ("d", "g")}
                for side in ("d", "g"):
                    # paired-subchunk stats + LN apply
                    mv4 = wp.tile([128, 4, 2], F32, tag="mv4", name="mv4",
                                  bufs=3)
                    xts = []
                    for pr in range(2):
                        xt = pp.tile([128, 2, 256], F16, tag="xt", name="xt",
                                     bufs=2)
                        xts.append(xt)
                        for jj in range(2):
                            j = 2 * pr + jj
                            for b in range(2):
                                src = (xup[:, b,
                                           off + 128 * j:off + 128 * j + 128]
                                       if side == "d"
                                       else rg[:, b, 128 * j:128 * j + 128])
                                nc.tensor.transpose(
                                    xt[:, jj, 128 * b:128 * b + 128], src,
                                    ident[:])
                        for jj in range(2):
                            st = wp.tile([128, 6], F32, tag="st", name="st",
                                         bufs=4)
                            nc.vector.bn_stats(st[:], xt[:, jj])
                            nc.vector.bn_aggr(mv4[:, 2 * pr + jj], st[:])
                    sq4 = wp.tile([128, 4], F32, tag="sq4", name="sq4",
                                  bufs=3)
                    nc.scalar.activation(sq4[:], mv4[:, :, 1], AF.Sqrt,
                                         bias=cst["epsb"][:])
                    av4 = wp.tile([128, 4], F32, tag="av4", name="av4",
                                  bufs=3)
                    nc.vector.reciprocal(av4[:], sq4[:])
                    cv4 = wp.tile([128, 4], F32, tag="cv4", name="cv4",
                                  bufs=3)
                    nc.vector.scalar_tensor_tensor(cv4[:], mv4[:, :, 0], -1.0,
                                                   av4[:], OP.mult, OP.mult)
                    lnv = ln[side].rearrange("p b (j x) -> p b j x", j=4)
                    for pr in range(2):
                        lnT = wp.tile([128, 2, 256], F16, tag="lnT",
                                      name="lnT", bufs=4)
                        for jj in range(2):
                            j = 2 * pr + jj
                            nc.vector.tensor_scalar(lnT[:, jj], xts[pr][:, jj],
                                                    av4[:, j:j + 1],
                                                    cv4[:, j:j + 1],
                                                    OP.mult, OP.add)
                        lnb = pp.tile([128, 512], F16, tag="tp", name="lnb",
                                      bufs=2)
                        for jj in range(2):
                            for b in range(2):
                                c0 = 256 * jj + 128 * b
                                nc.tensor.transpose(
                                    lnb[:, c0:c0 + 128],
                                    lnT[:, jj, 128 * b:128 * b + 128],
                                    ident[:])
                        nc.vector.tensor_copy(
                            lnv[:, :, 2 * pr:2 * pr + 2, :],
                            lnb[:].rearrange("p (jj b x) -> p b jj x", jj=2,
                                             b=2))
                # Q convs
                qsb = {}
                for side in ("d", "g"):
                    qps = [pp.tile([128, 512], F32, tag="cps",
                                   name=f"q{o}", bufs=2) for o in range(2)]
                    for o in range(2):
                        for kb in range(2):
                            nc.tensor.matmul(qps[o][:],
                                             cst[(f"wq_{side}", kb, o)][:],
                                             ln[side][:, kb], start=(kb == 0),
                                             stop=(kb == 1))
                    qt = wp.tile([128, 2, 512], F16, tag=f"q{side}",
                                 name=f"q{side}", bufs=2)
                    qsb[side] = qt
                    bcol = 0 if side == "d" else 2
                    sidx = 0 if side == "d" else 1
                    for o in range(2):
                        if o == 0:
                            nc.vector.tensor_scalar(
                                qt[:, o], qps[o][:],
                                cst["bq"][:, bcol + o:bcol + o + 1], None,
                                OP.add)
                        else:
                            nc.scalar.activation(
                                qt[:, o], qps[o][:], AF.Identity,
                                bias=cst["bq"][:, bcol + o:bcol + o + 1])
                        scr = wp.tile([128, 512], F16, tag="scr", name="scr",
                                      bufs=2)
                        nc.scalar.activation(
                            scr[:], qt[:, o], AF.Square, bias=cst["zb"][:],
                            accum_out=nacc[:, sidx, o, k:k + 1])
                    if "d_q" in dbg and side == "d":
                        for o in range(2):
                            nc.sync.dma_start(
                                dbg["d_q"][128 * o:128 * o + 128,
                                           512 * k:512 * k + 512], qt[:, o])
                    if "d_ln" in dbg and side == "d":
                        for b in range(2):
                            nc.sync.dma_start(
                                dbg["d_ln"][128 * b:128 * b + 128,
                                            512 * k:512 * k + 512],
                                ln[side][:, b])
                # V convs
                for side in ("d", "g"):
                    vps = [pp.tile([128, 512], F32, tag="cps",
                                   name=f"v{o}", bufs=2) for o in range(2)]
                    for o in range(2):
                        for kb in range(2):
                            rhs = (xup[:, kb, off:off + 512] if side == "d"
                                   else rg[:, kb])
                            nc.tensor.matmul(vps[o][:],
                                             cst[(f"wv_{side}", kb, o)][:],
                                             rhs, start=(kb == 0),
                                             stop=(kb == 1))
                    bcol = 0 if side == "d" else 2
                    for o in range(2):
                        nc.scalar.activation(
                            Vsb[side][:, o, off:off + 512], vps[o][:],
                            AF.Identity,
                            bias=cst["bv"][:, bcol + o:bcol + o + 1])
                # QT transposes + Gram
                for pr in range(2):
                    qtt = {}
                    for side in ("d", "g"):
                        qp = pp.tile([128, 2, 256], F16, tag="tp", name="qtp",
                                     bufs=2)
                        for jj in range(2):
                            j = 2 * pr + jj
                            for o in range(2):
                                nc.tensor.transpose(
                                    qp[:, jj, 128 * o:128 * o + 128],
                                    qsb[side][:, o, 128 * j:128 * j + 128],
                                    ident[:])
                        qs = wp.tile([128, 2, 256], F16, tag=f"qt{side}",
                                     name=f"qt{side}", bufs=2)
                        nc.vector.tensor_copy(qs[:], qp[:])
                        qtt[side] = qs
                    for jj in range(2):
                        j = 2 * pr + jj
                        for db in range(2):
                            nc.tensor.matmul(
                                Gps[db][:],
                                qtt["d"][:, jj, 128 * db:128 * db + 128],
                                qtt["g"][:, jj], start=(k == 0 and j == 0),
                                stop=(k == NCH - 1 and j == 3),
                                skip_group_check=True)

            # V convs for tile rows 0 and 65
            for off, cs in [(0, 128), (NPIX - 128, 128)]:
                rge = wp.tile([128, 2, 128], F16, tag="rge", name="rge",
                              bufs=2)
                for b in range(2):
                    nc.sync.dma_start(rge[:, b],
                                      res_in[128 * b:128 * b + 128,
                                             off:off + cs])
                for side in ("d", "g"):
                    vps = [pp.tile([128, 512], F32, tag="cps",
                                   name=f"ve{o}", bufs=2) for o in range(2)]
                    for o in range(2):
                        for kb in range(2):
                            rhs = (xup[:, kb, off:off + cs] if side == "d"
                                   else rge[:, kb])
                            nc.tensor.matmul(vps[o][:, 0:cs],
                                             cst[(f"wv_{side}", kb, o)][:],
                                             rhs, start=(kb == 0),
                                             stop=(kb == 1))
                    bcol = 0 if side == "d" else 2
                    for o in range(2):
                        nc.vector.tensor_scalar(
                            Vsb[side][:, o, off:off + cs], vps[o][:, 0:cs],
                            cst["bv"][:, bcol + o:bcol + o + 1], None, OP.add)
        if "d_v" in dbg:
            for b in range(2):
                nc.sync.dma_start(dbg["d_v"][128 * b:128 * b + 128, 0:NPIX],
                                  Vd[:, b])
                nc.sync.dma_start(dbg["d_v"][128 * b:128 * b + 128,
                                             NPIX:2 * NPIX], Vg[:, b])

        if stop_after < 2:
            _bail()
            return
        # ====================== phase 1.5: collective ======================
        with (
            tc.tile_pool(name="sft", bufs=1) as sp,
            tc.tile_pool(name="dram", bufs=1, space="DRAM") as dp,
            tc.tile_pool(name="sps", bufs=1, space="PSUM") as psS,
        ):
            nsum = sp.tile([128, 2, 2], F32, name="nsum")
            for sidx in range(2):
                for o in range(2):
                    nc.vector.tensor_reduce(nsum[:, sidx, o:o + 1],
                                            nacc[:, sidx, o], AX.X, OP.add)
            bounce_i = dp.tile([C, 260], F32, name="bounce_i")
            bounce_o = dp.tile([C, 260], F32, name="bounce_o")
            gsb = sp.tile([128, 2, 256], F32, name="gsb")
            for db in range(2):
                nc.vector.tensor_copy(gsb[:, db], Gps[db][:])
                nc.gpsimd.dma_start(bounce_i[128 * db:128 * db + 128, 0:256],
                                    gsb[:, db])
                nc.gpsimd.dma_start(bounce_i[128 * db:128 * db + 128,
                                             256:257],
                                    nsum[:, 0, db:db + 1])
                nc.gpsimd.dma_start(bounce_i[128 * db:128 * db + 128,
                                             257:258],
                                    nsum[:, 1, db:db + 1])
            nc.gpsimd.collective_compute(
                "AllReduce", OP.add,
                replica_groups=[[2 * i, 2 * i + 1]
                                for i in range(NCORES // 2)],
                ins=[bounce_i.opt()], outs=[bounce_o.opt()])

            if stop_after < 3:
                _bail()
                return
            # ====================== phase 1.6: softmax / A =================
            gr = sp.tile([128, 2, 256], F32, name="gr")
            ndr = sp.tile([128, 2], F32, name="ndr")
            ngr = sp.tile([128, 2], F32, name="ngr")
            for db in range(2):
                nc.sync.dma_start(gr[:, db],
                                  bounce_o[128 * db:128 * db + 128, 0:256])
                nc.sync.dma_start(ndr[:, db:db + 1],
                                  bounce_o[128 * db:128 * db + 128, 256:257])
                nc.sync.dma_start(ngr[:, db:db + 1],
                                  bounce_o[128 * db:128 * db + 128, 257:258])
            if "d_g" in dbg:
                for db in range(2):
                    nc.sync.dma_start(dbg["d_g"][128 * db:128 * db + 128, :],
                                      bounce_o[128 * db:128 * db + 128, :])
            rd = sp.tile([128, 2], F32, name="rd")
            sqn = sp.tile([128, 2], F32, name="sqn")
            nc.scalar.activation(sqn[:], ndr[:], AF.Sqrt, bias=cst["zb"][:])
            nc.vector.tensor_scalar(sqn[:], sqn[:], 1e-12, None, OP.max)
            nc.vector.reciprocal(rd[:], sqn[:])
            ngrow = sp.tile([4, 2, 32], F32, name="ngrow")
            for db in range(2):
                src = bounce_o[128 * db:128 * db + 128, 257:258].rearrange(
                    "(g j) one -> g (j one)", g=4)
                nc.sync.dma_start(ngrow[:, db], src)
            sq2 = sp.tile([4, 2, 32], F32, name="sq2")
            nc.scalar.activation(sq2[:], ngrow[:], AF.Sqrt,
                                 bias=cst["zb"][0:4, 0:1])
            nc.vector.tensor_scalar(sq2[:], sq2[:], 1e-12, None, OP.max)
            rg2 = sp.tile([4, 2, 32], F32, name="rg2")
            nc.vector.reciprocal(rg2[:], sq2[:])
            rg16 = sp.tile([4, 2, 32], F16, name="rg16")
            nc.vector.tensor_copy(rg16[:], rg2[:])

            for db in range(2):
                ngb = psS.tile([128, 32], F32, tag="ngb", name="ngb", bufs=2)
                nc.tensor.matmul(ngb[:], cst["bones"][:], rg16[:, db],
                                 start=True, stop=True)
                asm = sp.tile([128, 32], F32, name=f"asm{db}")
                for g in range(4):
                    h = 4 * db + g
                    nc.vector.tensor_copy(asm[32 * g:32 * g + 32, :],
                                          gr[32 * g:32 * g + 32, db,
                                             32 * h:32 * h + 32])
                t1 = sp.tile([128, 32], F32, name=f"t1{db}")
                nc.vector.tensor_scalar(t1[:], asm[:], rd[:, db:db + 1],
                                        cst["svec"][:, db:db + 1],
                                        OP.mult, OP.mult)
                nc.vector.tensor_tensor(t1[:], t1[:], ngb[:], OP.mult)
                mx = sp.tile([128, 1], F32, name=f"mx{db}")
                nc.vector.tensor_reduce(mx[:], t1[:], AX.X, OP.max)
                nmx = sp.tile([128, 1], F32, name=f"nmx{db}")
                nc.vector.tensor_scalar(nmx[:], mx[:], -1.0, None, OP.mult)
                ex = sp.tile([128, 32], F32, name=f"ex{db}")
                den = sp.tile([128, 1], F32, name=f"den{db}")
                nc.scalar.activation(ex[:], t1[:], AF.Exp, bias=nmx[:],
                                     accum_out=den[:])
                rec = sp.tile([128, 1], F32, name=f"rec{db}")
                nc.vector.reciprocal(rec[:], den[:])
                for ai, bcol in ((0, 0), (1, 2)):
                    sfac = sp.tile([128, 1], F32, name=f"sf{db}{ai}")
                    nc.vector.tensor_tensor(
                        sfac[:], rec[:],
                        cst["bgv"][:, bcol + db:bcol + db + 1], OP.mult)
                    a16 = sp.tile([128, 32], F16, name=f"a16{db}{ai}")
                    nc.vector.tensor_scalar(a16[:], ex[:], sfac[:], None,
                                            OP.mult)
                    if "d_a" in dbg:
                        nc.sync.dma_start(
                            dbg["d_a"][128 * db:128 * db + 128,
                                       32 * ai:32 * ai + 32], a16[:])
                    at = sp.tile([128, 32], F16, name=f"at{db}{ai}")
                    nc.vector.transpose(at[:], a16[:])
                    lt = bp.tile([128, 128], F16, tag=f"lA{db}{ai}",
                                 name=f"lA{db}{ai}")
                    nc.vector.memset(lt[:], 0.0)
                    for g in range(4):
                        nc.vector.tensor_copy(
                            lt[32 * g:32 * g + 32, 32 * g:32 * g + 32],
                            at[32 * g:32 * g + 32, :])
                    cst[("lA", db, ai)] = lt

    if stop_after < 4:
        _bail()
        return
    # ========================== phase 2: y assembly ========================
    ytile = bp.tile([128, 2, ROWS, YW], F16, tag="y", name="ytile")
    with (
        tc.tile_pool(name="p2ps", bufs=1, space="PSUM") as pp2,
        tc.tile_pool(name="p2w", bufs=1) as wp2,
    ):
        for o in range(2):
            nc.vector.memset(ytile[:, o, :, 0:1], 0.0)
            nc.vector.memset(ytile[:, o, :, 129:130], 0.0)
        for off, cs in _vchunks():
            rg = wp2.tile([128, 2, 512], F16, tag="rg2", name="rg2b", bufs=3)
            for b in range(2):
                nc.sync.dma_start(rg[:, b, 0:cs],
                                  res_in[128 * b:128 * b + 128, off:off + cs])
            for o in range(2):
                yp = pp2.tile([128, 512], F32, tag="yps", name=f"yp{o}",
                              bufs=3)
                nc.tensor.matmul(yp[:, 0:cs], cst[("lA", o, 0)][:],
                                 Vg[:, o, off:off + cs], start=True,
                                 stop=False)
                nc.tensor.matmul(yp[:, 0:cs], cst[("lA", o, 1)][:],
                                 Vd[:, o, off:off + cs], start=False,
                                 stop=False)
                nc.tensor.matmul(yp[:, 0:cs], ident[:],
                                 xup[:, o, off:off + cs], start=False,
                                 stop=False)
                nc.tensor.matmul(yp[:, 0:cs], ident[:], rg[:, o, 0:cs],
                                 start=False, stop=True)
                r0, nr = off // W2, cs // W2
                nc.vector.tensor_copy(
                    ytile[:, o, r0:r0 + nr, 1:129],
                    yp[:, 0:cs].rearrange("p (r w) -> p r w", w=W2))
        for o in range(2):
            nc.vector.tensor_scalar(ytile[:, o, 0, :], ytile[:, o, 0, :],
                                    cst["msk"][:, 0:1], None, OP.mult)
            nc.vector.tensor_scalar(ytile[:, o, 65, :], ytile[:, o, 65, :],
                                    cst["msk"][:, 1:2], None, OP.mult)
        if "d_y" in dbg:
            for o in range(2):
                nc.sync.dma_start(dbg["d_y"][128 * o:128 * o + 128, :],
                                  ytile[:, o].rearrange("p r w -> p (r w)"))

    if stop_after < 5:
        _bail()
        return
    # ========================== phase 3: DW + PW ===========================
    with (
        tc.tile_pool(name="p3ps", bufs=1, space="PSUM") as pp3,
        tc.tile_pool(name="p3w", bufs=1) as wp3,
    ):
        for grp in range(4):
            dwc = wp3.tile([128, 2, 4, 512], F16, tag="dwc", name="dwc",
                           bufs=2)
            for o in range(2):
                pd = [pp3.tile([128, 512], F32, tag="pd", name=f"pd{ci}",
                               bufs=4) for ci in range(4)]
                for s in range(9):
                    dr, dc = s // 3 - 1, s % 3 - 1
                    for ci in range(4):
                        r0 = 1 + 4 * (4 * grp + ci) + dr
                        rhs = (ytile[:, o, r0:r0 + 4, 1 + dc:129 + dc]
                               if stop_after != 8 else
                               ytile[:, o, r0:r0 + 4, 1:129])
                        nc.tensor.matmul(
                            pd[ci][:], cst[("dw", s, o)][:], rhs,
                            start=(s == 0), stop=(s == 8),
                            skip_group_check=True)
                for ci in range(4):
                    nc.vector.tensor_copy(dwc[:, o, ci], pd[ci][:])
            if stop_after == 5:
                for ci in range(4):
                    for o in range(2):
                        osb = wp3.tile([128, 512], F32, tag="osb", name="osb",
                                       bufs=3)
                        nc.vector.tensor_copy(osb[:], dwc[:, o, ci])
                        nc.sync.dma_start(
                            out_d[128 * o:128 * o + 128,
                                  512 * (4 * grp + ci):512 * (4 * grp + ci) + 512],
                            osb[:])
                continue
            for ci in range(4):
                chunk = 4 * grp + ci
                for o in range(2):
                    po = pp3.tile([128, 512], F32, tag="po", name=f"po{o}",
                                  bufs=2)
                    for kb in range(2):
                        nc.tensor.matmul(po[:], cst[("pw_l", kb, o)][:],
                                         dwc[:, kb, ci], start=(kb == 0),
                                         stop=(kb == 1))
                    osb = wp3.tile([128, 512], F32, tag="osb", name="osb",
                                   bufs=3)
                    nc.vector.tensor_scalar(osb[:], po[:],
                                            cst["pwc"][:, o:o + 1], 0.0,
                                            OP.add, OP.max)
                    nc.vector.tensor_scalar(osb[:], osb[:], 6.0, None,
                                            OP.min)
                    nc.sync.dma_start(
                        out_d[128 * o:128 * o + 128,
                              512 * chunk:512 * chunk + 512], osb[:])


# ------------------------------------------------------------- host side --
_CACHE = {}


def _fold(inp):
    f = {}
    f['wq_d'] = inp['w_dp1'] * inp['ln_d_w'][None, :]
    f['bq_d'] = inp['w_dp1'] @ inp['ln_d_b'] + inp['b_dp1']
    f['wq_g'] = inp['w_gp1'] * inp['ln_g_w'][None, :]
    f['bq_g'] = inp['w_gp1'] @ inp['ln_g_b'] + inp['b_gp1']
    bn_sc = inp['bn_g'] / np.sqrt(inp['bn_v'] + 1e-5)
    f['pw'] = inp['pw_w'] * bn_sc[None, :]
    f['pwc'] = inp['pw_w'] @ (inp['bn_b'] - inp['bn_m'] * bn_sc)
    f['dw'] = inp['dw_w'][:, 0]
    f['scale'] = np.repeat(np.asarray(inp['scale']).reshape(HEADS),
                           C // HEADS)
    f['beta'] = inp['beta'].reshape(C)
    f['gamma'] = inp['gamma'].reshape(C)
    return f


def _lhsT_blocks(wmat):
    """[o,c] weight -> [kblk, oblk, 128, 128] f16 lhsT (lhsT[k=c, m=o])."""
    wt = wmat.T.astype(np.float16)
    out = np.zeros((2, 2, 128, 128), np.float16)
    for kb in range(2):
        for ob in range(2):
            out[kb, ob] = wt[128 * kb:128 * kb + 128,
                             128 * ob:128 * ob + 128]
    return out


def _pair_cols(a, b):
    """two [C] vectors -> [128, 4] (a_blk0, a_blk1, b_blk0, b_blk1)."""
    out = np.zeros((128, 4), np.float32)
    out[:, 0] = a[0:128]
    out[:, 1] = a[128:256]
    out[:, 2] = b[0:128]
    out[:, 3] = b[128:256]
    return out


def _core_inputs(inp, f, shared, core):
    s, half = core // 2, core % 2
    r0 = 64 * half
    x, res = np.asarray(inp['x']), np.asarray(inp['res'])
    rs = np.zeros((C, ROWS, W2), np.float32)
    lo, hi = r0 - 1, r0 + 65
    slo, shi = max(lo, 0), min(hi, H2)
    rs[:, slo - lo:shi - lo, :] = res[s, :, slo:shi, :]
    rows = np.clip(np.arange(XR) + r0 // 2 - 1, 0, H - 1)
    xs = x[s][:, rows, :]
    m_top = 0.0 if half == 0 else 1.0
    m_bot = 0.0 if half == 1 else 1.0
    d = dict(shared)
    d["xs"] = xs.astype(np.float16).reshape(C, XR * W)
    d["res"] = rs.astype(np.float16).reshape(C, NPIX)
    d["msk"] = np.tile(np.array([[m_top, m_bot]], np.float32), (128, 1))
    return d


def _shared_inputs(inp, f):
    dwdiag = np.zeros((9, 2, 128, 128), np.float16)
    for srow in range(3):
        for scol in range(3):
            sidx = 3 * srow + scol
            for ob in range(2):
                v = f['dw'][128 * ob:128 * ob + 128, srow, scol]
                np.fill_diagonal(dwdiag[sidx, ob], v.astype(np.float16))
    bones = np.zeros((4, 128), np.float16)
    for g in range(4):
        bones[g, 32 * g:32 * g + 32] = 1.0
    col2 = lambda v: np.stack([v[0:128], v[128:256]], 1).astype(np.float32)
    return {
        "wq_d": _lhsT_blocks(f['wq_d']), "wq_g": _lhsT_blocks(f['wq_g']),
        "wv_d": _lhsT_blocks(np.asarray(inp['w_dp2'])),
        "wv_g": _lhsT_blocks(np.asarray(inp['w_gp2'])),
        "pw_l": _lhsT_blocks(f['pw']),
        "dwd": dwdiag,
        "ident": np.eye(128, dtype=np.float16),
        "bones": bones,
        "bq": _pair_cols(f['bq_d'], f['bq_g']),
        "bv": _pair_cols(np.asarray(inp['b_dp2']), np.asarray(inp['b_gp2'])),
        "pwc": col2(f['pwc']),
        "svec": col2(f['scale']),
        "bgv": _pair_cols(f['beta'], f['gamma']),
    }


def make_runner(nc, n_cores=NCORES):
    import jax
    from jax.sharding import Mesh, PartitionSpec, NamedSharding
    from jax.experimental.shard_map import shard_map
    from concourse.bass2jax import (_bass_exec_p, install_neuronx_cc_hook,
                                    partition_id_tensor)
    install_neuronx_cc_hook()
    pname = nc.partition_id_tensor.name if nc.partition_id_tensor else None
    in_names, out_names, out_avals, zero_outs = [], [], [], []
    for alloc in nc.m.functions[0].allocations:
        if not isinstance(alloc, mybir.MemoryLocationSet):
            continue
        name = alloc.memorylocations[0].name
        if alloc.kind == "ExternalInput":
            if name != pname:
                in_names.append(name)
        elif alloc.kind == "ExternalOutput":
            shape = tuple(alloc.tensor_shape)
            dd = mybir.dt.np(alloc.dtype)
            out_names.append(name)
            out_avals.append(jax.core.ShapedArray(shape, dd))
            zero_outs.append(np.zeros(shape, dd))
    n_params, n_outs = len(in_names), len(out_avals)
    all_in = in_names + out_names + ([pname] if pname else [])
    donate = tuple(range(n_params, n_params + n_outs))

    def _body(*args):
        operands = list(args)
        if pname is not None:
            operands.append(partition_id_tensor())
        return tuple(_bass_exec_p.bind(
            *operands, out_avals=tuple(out_avals), in_names=tuple(all_in),
            out_names=tuple(out_names), lowering_input_output_aliases=(),
            sim_require_finite=False, sim_require_nnan=False, nc=nc))

    devices = jax.devices()[:n_cores]
    mesh = Mesh(np.asarray(devices), ("core",))
    sharded = jax.jit(
        shard_map(_body, mesh=mesh,
                  in_specs=(PartitionSpec("core"),) * (n_params + n_outs),
                  out_specs=(PartitionSpec("core"),) * n_outs,
                  check_rep=False),
        donate_argnums=donate, keep_unused=True)
    sharding = NamedSharding(mesh, PartitionSpec("core"))

    def run(in_maps, n_timed=0):
        import time as _t
        per_core = [[np.asarray(m[n]) for n in in_names] for m in in_maps]
        concat_in = [np.concatenate([per_core[c][i] for c in range(n_cores)],
                                    0) for i in range(n_params)]
        dev_in = [jax.device_put(a, sharding) for a in concat_in]

        def zeros():
            return [jax.device_put(
                np.zeros((n_cores * z.shape[0], *z.shape[1:]), z.dtype),
                sharding) for z in zero_outs]

        out = sharded(*dev_in, *zeros())
        jax.block_until_ready(out)
        times = []
        for _ in range(n_timed):
            zs = zeros()
            jax.block_until_ready(zs)
            t0 = _t.perf_counter()
            out2 = sharded(*dev_in, *zs)
            jax.block_until_ready(out2)
            times.append(_t.perf_counter() - t0)
            out = out2
        results = [
            {n: np.asarray(out[i]).reshape(n_cores, *out_avals[i].shape)[c]
             for i, n in enumerate(out_names)} for c in range(n_cores)]
        return results, times

    return run


def _get(reps=1, debug=False):
    key = (reps, debug)
    if key not in _CACHE:
        nc = build(reps=reps, debug=debug)
        _CACHE[key] = (nc, make_runner(nc))
    return _CACHE[key]


def kernel(**inputs) -> np.ndarray:
    inp = {k: np.asarray(v) for k, v in inputs.items()}
    f = _fold(inp)
    shared = _shared_inputs(inp, f)
    in_maps = [_core_inputs(inp, f, shared, core) for core in range(NCORES)]
    nc, run = _get()
    results, _ = run(in_maps)
    out = np.zeros((B, C, H2, W2), np.float32)
    for core in range(NCORES):
        s, half = core // 2, core % 2
        out[s, :, 64 * half:64 * half + 64, :] = \
            results[core]["out"].reshape(C, 64, W2)
    return out


if __name__ == "__main__":
    import reference as R
    inp = {k: np.asarray(v) for k, v in R.setup_inputs().items()}
    exp = np.asarray(R.reference(**R.setup_inputs()))
    got = kernel(**inp)
    rel = np.linalg.norm(got - exp) / np.linalg.norm(exp)
    print("kernel rel l2:", rel, " max abs:", np.abs(got - exp).max())

